# revision 27
# baseline (speedup 1.0000x reference)
"""Multi-head self-attention (B=4, S=2048, D=768, H=12) on 8 Trainium2 cores.

Sharding: data-parallel over the 4 batches x tensor-parallel over 2 head-groups
of 6 heads (12 heads, QKV column-split / out-proj row-split). Each core gets the
full query range of its batch but only the mask-unmasked keys (host-compacted,
padded to S_KV=1152): masked keys contribute exactly 0 after softmax, so
dropping them halves the attention work. Per-core outputs are partial
out-projections; the host sums each batch's two partials and adds bo.

Device layouts keep the contraction dim on partitions everywhere so no on-chip
transpose is ever needed:
  qT/kT [384, S*]   feature-on-partition (head h = partitions 64h..64h+63)
  v'    [S_KV, 390] key-on-partition, per-head 65-col group = [v_h | 1]; the
                    ones column makes the PV matmul also emit the softmax
                    denominator as out row 64
  scoresT [k, q]    exp runs on ScalarE with the padded-key -1e9 bias folded
                    into the per-partition activation bias; temperature/sqrt(d)
                    is pre-multiplied into qT via a per-partition scale vector
                    (input data, keeping the program SPMD-uniform)
"""

import math

import numpy as np

import concourse.bass as bass
import concourse.mybir as mybir
import concourse.tile as tile
from concourse.bass_utils import run_bass_kernel_spmd

F32 = mybir.dt.float32
F32R = mybir.dt.float32r


def _r(ap):
    """Bitcast an fp32 AP to float32r: single-pass PE matmul (4x faster,
    slightly reduced multiply precision)."""
    return ap.bitcast(F32R)
AF = mybir.ActivationFunctionType
ALU = mybir.AluOpType

D_MODEL = 768
NUM_HEADS = 12
D_QKV = 64
B = 4
S = 2048
N_CORES = 8
HPC = NUM_HEADS // 2          # heads per core = 6
M = HPC * D_QKV               # per-core feature slice = 384
S_KV = 1152                   # padded compacted key count (9 * 128)
KB_KV = S_KV // 128           # 9
KB_D = D_MODEL // 128         # 6
PB = M // 128                 # 3 feature partition-blocks per core

_PROGRAM = None


def _split_wide_waits(nc, max_waits=1):
    """walrus core_v3 codegen rejects >2 semaphore waits on one instruction
    (hit by the Tile-exit Drain). Hoist excess waits onto NoOps inserted just
    before, on the same engine stream — sequential waits are equivalent."""
    for fn in nc.m.functions:
        for blk in fn.blocks:
            insts = blk.instructions
            i = 0
            while i < len(insts):
                inst = insts[i]
                si = inst.sync_info
                if si is not None and len(si.on_wait) > max_waits:
                    waits = list(si.on_wait)
                    keep, rest = waits[:max_waits], waits[max_waits:]
                    k = 0
                    while rest:
                        chunk, rest = rest[:max_waits], rest[max_waits:]
                        # a real Drain (not NoOp: walrus folds NoOps into the
                        # successor, recombining the waits)
                        nop = mybir.InstDrain(
                            name=f"{inst.name}_wsplit{k}", ins=[], outs=[]
                        )
                        nop.engine = inst.engine
                        nop.is_reset_sema = False
                        nop.sync_info = mybir.SyncInfo(on_wait=chunk, on_update=[])
                        insts.insert(i, nop)
                        i += 1
                        k += 1
                    inst.sync_info = mybir.SyncInfo(
                        on_wait=keep, on_update=list(si.on_update)
                    )
                i += 1


def _build_program():
    nc = bass.Bass("TRN2", target_bir_lowering=False, debug=False)

    def din(name, shape, dt=F32):
        return nc.dram_tensor(name, list(shape), dt, kind="ExternalInput").ap()

    xT_d = din("xT", [D_MODEL, S], F32R)
    xcT_d = din("xcT", [D_MODEL, S_KV], F32R)
    wqT_d = din("wqT", [D_MODEL, M], F32R)
    wkT_d = din("wkT", [D_MODEL, M], F32R)
    wvT_d = din("wvT", [D_MODEL, M], F32R)
    woT_d = din("woT", [M, D_MODEL], F32R)
    bq_d = din("bq", [PB, 128])
    bk_d = din("bk", [PB, 128])
    sq_d = din("sq", [PB, 128])        # per-partition q scale = temp[h]/sqrt(d)
    bvb_d = din("bvb", [128, M])       # bv broadcast across partitions
    kbias_d = din("kbias", [KB_KV, 128, HPC])  # exp bias: 0 real, -1e9*s pad
    ones_d = din("ones", [128, HPC], F32R)     # the v' ones columns
    out_d = nc.dram_tensor("out", [S, D_MODEL], F32, kind="ExternalOutput").ap()

    with tile.TileContext(nc) as tc:
        with (
            tc.tile_pool(name="wpool", bufs=1) as wpool,
            tc.tile_pool(name="bigp", bufs=6) as bigp,
            tc.tile_pool(name="midp", bufs=6) as midp,
            tc.tile_pool(name="perp", bufs=1) as perp,
            tc.tile_pool(name="obp", bufs=2) as obp,
            tc.tile_pool(name="psp", bufs=2, space="PSUM") as psp,
        ):
            # ---- load weights / small constants --------------------------
            def load(pool, dram, shape, name, tag, bufs=None, dt=F32R):
                t = pool.tile(list(shape), dt, name=name, tag=tag, bufs=bufs)
                nc.sync.dma_start(out=t[:], in_=dram)
                return t

            wqT = [
                load(wpool, wqT_d[kb * 128 : (kb + 1) * 128, :], [128, M],
                     f"wqT{kb}", f"wqT{kb}")
                for kb in range(KB_D)
            ]
            wkT = [
                load(wpool, wkT_d[kb * 128 : (kb + 1) * 128, :], [128, M],
                     f"wkT{kb}", f"wkT{kb}")
                for kb in range(KB_D)
            ]
            wvT = [
                load(wpool, wvT_d[kb * 128 : (kb + 1) * 128, :], [128, M],
                     f"wvT{kb}", f"wvT{kb}")
                for kb in range(KB_D)
            ]
            woT = [
                load(wpool, woT_d[pb * 128 : (pb + 1) * 128, :], [128, D_MODEL],
                     f"woT{pb}", f"woT{pb}")
                for pb in range(PB)
            ]
            bvb = load(wpool, bvb_d, [128, M], "bvb", "bvb", dt=F32)
            onescol = wpool.tile([128, 64], F32, name="onescol", tag="onescol")
            nc.vector.memset(onescol[:], 1.0)
            kbias = wpool.tile([128, KB_KV * HPC], F32, name="kbias", tag="kbias")
            for kb in range(KB_KV):
                nc.sync.dma_start(
                    out=kbias[:, kb * HPC : (kb + 1) * HPC], in_=kbias_d[kb]
                )
            bq = wpool.tile([128, PB], F32, name="bq", tag="bq")
            bk = wpool.tile([128, PB], F32, name="bk", tag="bk")
            sq = wpool.tile([128, PB], F32, name="sq", tag="sq")
            for pb in range(PB):
                nc.sync.dma_start(out=bq[:, pb : pb + 1], in_=bq_d[pb, :, None])
                nc.sync.dma_start(out=bk[:, pb : pb + 1], in_=bk_d[pb, :, None])
                nc.sync.dma_start(out=sq[:, pb : pb + 1], in_=sq_d[pb, :, None])

            # ---- load xT (shares "big" slots with attT later) ------------
            xT = [
                load(bigp, xT_d[kb * 128 : (kb + 1) * 128, :], [128, S],
                     f"xT{kb}", "big", bufs=6)
                for kb in range(KB_D)
            ]
            # xcT shares "mid" slots with pT/den later
            xcT = [
                load(midp, xcT_d[kb * 128 : (kb + 1) * 128, :], [128, S_KV],
                     f"xcT{kb}", "mid", bufs=6)
                for kb in range(KB_D)
            ]

            qT = [
                perp.tile([128, S], F32R, name=f"qT{pb}", tag=f"qT{pb}")
                for pb in range(PB)
            ]
            kT = [
                perp.tile([128, S_KV], F32R, name=f"kT{pb}", tag=f"kT{pb}")
                for pb in range(PB)
            ]
            vp = [
                perp.tile([128, HPC * 65], F32R, name=f"vp{sb}", tag=f"vp{sb}")
                for sb in range(KB_KV)
            ]

            # ---- phase 1: qT = (wqT.T @ xT + bq) * s ---------------------
            for pb in range(PB):
                for qb in range(S // 512):
                    ps = psp.tile([128, 512], F32, name="mmq", tag="mm")
                    for kb in range(KB_D):
                        nc.tensor.matmul(
                            ps[:],
                            lhsT=wqT[kb][:, pb * 128 : (pb + 1) * 128],
                            rhs=xT[kb][:, qb * 512 : (qb + 1) * 512],
                            start=(kb == 0),
                            stop=(kb == KB_D - 1),
                        )
                    nc.vector.tensor_scalar(
                        out=qT[pb][:, qb * 512 : (qb + 1) * 512],
                        in0=ps[:],
                        scalar1=bq[:, pb : pb + 1],
                        scalar2=sq[:, pb : pb + 1],
                        op0=ALU.add,
                        op1=ALU.mult,
                    )

            # ---- phase 2: kT = wkT.T @ xcT + bk --------------------------
            for pb in range(PB):
                for cb in range(S_KV // 384):
                    ps = psp.tile([128, 384], F32, name="mmk", tag="mm")
                    for kb in range(KB_D):
                        nc.tensor.matmul(
                            ps[:],
                            lhsT=wkT[kb][:, pb * 128 : (pb + 1) * 128],
                            rhs=xcT[kb][:, cb * 384 : (cb + 1) * 384],
                            start=(kb == 0),
                            stop=(kb == KB_D - 1),
                        )
                    nc.vector.tensor_scalar_add(
                        kT[pb][:, cb * 384 : (cb + 1) * 384],
                        ps[:],
                        bk[:, pb : pb + 1],
                    )

            # ---- phase 3: v' = [xc @ wvT + bv | 1] -----------------------
            for sb in range(KB_KV):
                ps = psp.tile([128, 384], F32, name="mmv", tag="mm")
                for kb in range(KB_D):
                    nc.tensor.matmul(
                        ps[:],
                        lhsT=xcT[kb][:, sb * 128 : (sb + 1) * 128],
                        rhs=wvT[kb][:, 0:M],
                        start=(kb == 0),
                        stop=(kb == KB_D - 1),
                    )
                dst = vp[sb].rearrange("p (h c) -> p h c", c=65)[:, :, 0:64]
                nc.vector.tensor_add(
                    dst,
                    ps.rearrange("p (h c) -> p h c", c=64),
                    bvb.rearrange("p (h c) -> p h c", c=64),
                )
                ones_col = vp[sb].rearrange("p (h c) -> p h c", c=65)[:, :, 64:65]
                nc.sync.dma_start(out=ones_col, in_=ones_d[:, :, None])

            # attT shares the "big" slots freed by xT
            attT = [
                bigp.tile([128, S], F32R, name=f"attT{pb}", tag="big", bufs=6)
                for pb in range(PB)
            ]
            # 1/denominator rows: 12 slots of [1, 1024] packed on the 3 legal
            # matmul base partitions (0/32/64) x 4 column slots
            rden = perp.tile([128, 4 * 1024], F32, name="rden", tag="rden")

            def rden_ap(slot, lo, hi):
                p = 32 * (slot % 3)
                c = (slot // 3) * 1024
                return rden[p : p + 1, c + lo : c + hi]

            # ---- phase 4: per (head, q-half): scoresT -> exp -> PV -------
            for h in range(HPC):
                pb, po = h // 2, 64 * (h % 2)
                for qh in range(2):
                    op = psp.tile([65, 1024], F32, name="outp", tag="outp")
                    for kb in range(KB_KV):
                        sc = psp.tile([128, 1024], F32, name="sc", tag="mm")
                        for nb in range(2):
                            nc.tensor.matmul(
                                sc[:, nb * 512 : (nb + 1) * 512],
                                lhsT=kT[pb][po : po + 64, kb * 128 : (kb + 1) * 128],
                                rhs=qT[pb][
                                    po : po + 64,
                                    qh * 1024 + nb * 512 : qh * 1024 + (nb + 1) * 512,
                                ],
                                start=True,
                                stop=True,
                            )
                        pt = midp.tile([128, 1024], F32R, name="pt", tag="mid", bufs=6)
                        nc.scalar.activation(
                            pt[:],
                            sc[:],
                            AF.Exp,
                            bias=kbias[:, kb * HPC + h : kb * HPC + h + 1],
                            scale=1.0,
                        )
                        for nb in range(2):
                            nc.tensor.matmul(
                                op[:, nb * 512 : (nb + 1) * 512],
                                lhsT=vp[kb][:, h * 65 : h * 65 + 65],
                                rhs=pt[:, nb * 512 : (nb + 1) * 512],
                                start=(kb == 0),
                                stop=(kb == KB_KV - 1),
                            )
                    # softmax 1/denominator -> partition-0 flat row;
                    # unnormalized att rows -> attT
                    slot = 2 * h + qh
                    nc.vector.reciprocal(rden_ap(slot, 0, 1024), op[64:65, :])
                    nc.vector.tensor_copy(
                        attT[pb][po : po + 64, qh * 1024 : (qh + 1) * 1024],
                        op[0:64, :],
                    )

            # ---- phase 5: normalize: attT *= bcast(1/den) ----------------
            # K=1 matmuls broadcast each partition-0 recip row to the 64
            # partitions of its head (col tile_position packs head pairs).
            for pb in range(PB):
                for qh in range(2):
                    bc = psp.tile([128, 1024], F32, name="bc", tag="mm")
                    for hh in range(2):  # head within the pair
                        slot = 2 * (2 * pb + hh) + qh
                        p = 32 * (slot % 3)
                        for nb in range(2):
                            nc.tensor.matmul(
                                bc[hh * 64 : hh * 64 + 64, nb * 512 : (nb + 1) * 512],
                                lhsT=onescol[p : p + 1, 0:64],
                                rhs=rden_ap(slot, nb * 512, (nb + 1) * 512),
                                start=True,
                                stop=True,
                            )
                    nc.vector.tensor_mul(
                        attT[pb][:, qh * 1024 : (qh + 1) * 1024],
                        attT[pb][:, qh * 1024 : (qh + 1) * 1024],
                        bc[:],
                    )

            # ---- phase 6: out = attT.T @ woT (partial; host adds pair+bo) -
            for sb in range(S // 128):
                ps = psp.tile([128, D_MODEL], F32, name="mmo", tag="mm")
                for pb in range(PB):
                    for lo, hi in ((0, 512), (512, 768)):
                        nc.tensor.matmul(
                            ps[:, lo:hi],
                            lhsT=attT[pb][:, sb * 128 : (sb + 1) * 128],
                            rhs=woT[pb][:, lo:hi],
                            start=(pb == 0),
                            stop=(pb == PB - 1),
                        )
                ob = obp.tile([128, D_MODEL], F32, name="ob", tag="ob")
                nc.scalar.copy(ob[:], ps[:])
                nc.sync.dma_start(
                    out=out_d[sb * 128 : (sb + 1) * 128, :], in_=ob[:]
                )

    _split_wide_waits(nc)
    return nc


def _prep_core_inputs(x, mask, Wq, bq, Wk, bk, Wv, bv, Wo, bo, temperature):
    """Build the 8 per-core input dicts (host-side shard + compact)."""
    scale = temperature.astype(np.float64) / math.sqrt(D_QKV)  # [12]
    in_maps = []
    for core in range(N_CORES):
        b, g = core // 2, core % 2
        sl = slice(g * M, (g + 1) * M)
        heads = slice(g * HPC, (g + 1) * HPC)

        xT = np.ascontiguousarray(x[b].T)  # [768, 2048]
        idx = np.flatnonzero(mask[b] != 0)
        nk = idx.size
        assert nk <= S_KV, f"batch {b}: {nk} unmasked keys > S_KV={S_KV}"
        xc = np.zeros((S_KV, D_MODEL), np.float32)
        xc[:nk] = x[b][idx]
        xcT = np.ascontiguousarray(xc.T)

        s_h = scale[heads].astype(np.float32)  # [6]
        # per-feature-partition scale for qT ([3,128]: feature -> head/64)
        sq = np.repeat(s_h, D_QKV).reshape(PB, 128)
        # exp bias: 0 for real keys, -1e9 * s_h for padded keys
        kbias = np.zeros((S_KV, HPC), np.float32)
        kbias[nk:, :] = -1e9 * s_h[None, :]
        kbias = np.ascontiguousarray(kbias.reshape(KB_KV, 128, HPC))

        in_maps.append(
            {
                "xT": xT,
                "xcT": xcT,
                "wqT": np.ascontiguousarray(Wq[sl, :].T),
                "wkT": np.ascontiguousarray(Wk[sl, :].T),
                "wvT": np.ascontiguousarray(Wv[sl, :].T),
                "woT": np.ascontiguousarray(Wo[:, sl].T),
                "bq": np.ascontiguousarray(bq[sl].reshape(PB, 128)),
                "bk": np.ascontiguousarray(bk[sl].reshape(PB, 128)),
                "sq": np.ascontiguousarray(sq),
                "bvb": np.broadcast_to(bv[sl], (128, M)).copy(),
                "ones": np.ones((128, HPC), np.float32),
                "kbias": kbias,
            }
        )
    return in_maps


def kernel(x, mask, Wq, bq, Wk, bk, Wv, bv, Wo, bo, temperature, **kw):
    global _PROGRAM
    x = np.asarray(x, np.float32)
    mask = np.asarray(mask)
    args = [np.asarray(a, np.float32) for a in (Wq, bq, Wk, bk, Wv, bv, Wo, bo)]
    temperature = np.asarray(temperature, np.float32)

    if _PROGRAM is None:
        _PROGRAM = _build_program()
    nc = _PROGRAM

    in_maps = _prep_core_inputs(x, mask, *args, temperature)
    res = run_bass_kernel_spmd(nc, in_maps, core_ids=list(range(N_CORES)))

    bo_f = args[7]
    out = np.empty((B, S, D_MODEL), np.float32)
    for b in range(B):
        out[b] = res.results[2 * b]["out"] + res.results[2 * b + 1]["out"] + bo_f
    return out
